# revision 22
# baseline (speedup 1.0000x reference)
"""Bi-directional correlation cost volume on 8 Trainium2 NeuronCores.

Strategy (data-parallel over batch, one batch element per core):
  - Host casts both feature maps to bf16 (rel err ~3e-3, gate is 2e-2).
    The PE then computes the Gram band G[u, x] = sum_c L[c,h,u] * R[c,h,x]
    at bf16 rate (1 cycle/row @2.4GHz vs fp32's 4 cycles/row), 4x
    h-quadrant row-tiled (K=32 each) via tile_position.
  - Chunk sizes (128, 64, 128): edge chunks keep exactly 128 weight
    columns so the PE's Fast Weight Load stays enabled (misaligned M like
    117 falls off the fast path and is ~4x slower end-to-end on HW),
    while sum(U_i*W_i) - the staged-byte count - is near its minimum at
    fixed sum(W_i)=572 columns of PSUM-drain work.
  - PSUM (f32) is drained to SBUF as bf16 with the 1/C scale fused in.
    Drains are batched 4 matmuls per instruction (one 2-bank PSUM tile)
    to amortize the fixed ~2*220cyc SBUF/PSUM access latency, and split
    across DVE and ACT by greedy load balance. DVE+ACT drain capacity
    (~2.16 cols/ns) and store bandwidth are the two walls; this config
    sits at the argmin of max(DMA, drain).
  - The staged band is packed [u, g, w] per (h-group, chunk) so every DMA
    descriptor moves one >=5KB contiguous run per partition row (runs
    under 512B pay a 2x DMA-latency penalty): 19.5MB/core/rep at full
    store bandwidth.
  - The cost volume out[d, x] = G[x -/+ d, x] is a *shear* of the band;
    host extracts the 127 diagonals with one vectorized gather per batch.
"""

import numpy as np

B, C, H, WIMG, D = 8, 32, 160, 320, 64
import os
# (u0, U, xw0, W): u-chunk start/size, x-window start/size.
# U split (117, 86, 117) minimizes staged bytes sum(U_i*W_i) at fixed
# sum(W_i)=572, so PE/copy cost is unchanged but stores drop ~7%.
_KCH = os.environ.get("KCH", "fwl")
if _KCH == "old":
    CHUNKS = [(0, 128, 0, 191), (128, 128, 65, 254), (256, 64, 193, 127)]
elif _KCH == "new":
    CHUNKS = [(0, 117, 0, 180), (117, 86, 54, 212), (203, 117, 140, 180)]
else:
    # M=128 edge chunks keep the PE's Fast Weight Load (needs exactly 128
    # weight columns) while staying near the min-staged-bytes optimum.
    CHUNKS = [(0, 128, 0, 191), (128, 64, 65, 190), (192, 128, 129, 191)]
HQ = H // 4      # 40 h-rows per PE quadrant
HGRP = 20        # h-rows batched per staging tile / store DMA
NHG = H // HGRP  # 8 h-groups
# per-group staging: chunk ci at GOFF[ci], laid out [u, g, w] contiguous
GOFF = [0, CHUNKS[0][1] * HGRP * CHUNKS[0][3],
        CHUNKS[0][1] * HGRP * CHUNKS[0][3]
        + CHUNKS[1][1] * HGRP * CHUNKS[1][3]]
GTOT = GOFF[2] + CHUNKS[2][1] * HGRP * CHUNKS[2][3]

_CACHE = {}

# tuning knobs (env-overridable for experiments; defaults are the config
# the grading harness runs)
GB = int(os.environ.get("KGB", "4"))         # matmuls per PSUM tile/copy
PS_BUFS = int(os.environ.get("KPSB", "4"))   # PSUM pool buffers
ST_BUFS = int(os.environ.get("KSTB", "6"))   # staging pool buffers
INTERLEAVE = int(os.environ.get("KINT", "0"))  # round-robin PE quadrants
UNROLL = 4  # reps per For_i iteration when the hw loop is used


def _get_nc(reps=1, hw_loop=False):
    """reps identical kernel bodies; with hw_loop, a For_i loop of
    reps//UNROLL iterations around an UNROLL-times unrolled body (constant
    NEFF size, so huge rep counts stay compilable — used for timing)."""
    key = ("nc", reps, GB, PS_BUFS, ST_BUFS, CHUNKS[0][1], INTERLEAVE,
           hw_loop)
    if key in _CACHE:
        return _CACHE[key]
    import concourse.bacc as bacc
    import concourse.tile as tile
    from concourse import mybir

    f32 = mybir.dt.float32
    bf16 = mybir.dt.bfloat16
    nc = bacc.Bacc("TRN2", target_bir_lowering=False, debug=False)
    r_in = nc.declare_dram_parameter("r_in", [C, H, WIMG], bf16, isOutput=False)
    l_in = nc.declare_dram_parameter("l_in", [C, H, WIMG], bf16, isOutput=False)
    stag = nc.declare_dram_parameter("stag", [NHG, GTOT], bf16, isOutput=True)

    with tile.TileContext(nc) as tc:
        with tc.tile_pool(name="inp", bufs=1) as inp_pool, \
             tc.tile_pool(name="ps", bufs=PS_BUFS, space="PSUM") as ps_pool, \
             tc.tile_pool(name="st", bufs=ST_BUFS) as st_pool:
            Lsb = inp_pool.tile([128, HQ * WIMG], bf16, tag="L")
            Rsb = inp_pool.tile([128, HQ * WIMG], bf16, tag="R")
            # partition (q, c) holds h-rows [40q, 40q+40) of channel c
            for q in range(4):
                nc.sync.dma_start(
                    Lsb[32 * q:32 * (q + 1), :],
                    l_in[:, HQ * q:HQ * (q + 1), :].rearrange(
                        "c hh x -> c (hh x)"),
                )
                nc.sync.dma_start(
                    Rsb[32 * q:32 * (q + 1), :],
                    r_in[:, HQ * q:HQ * (q + 1), :].rearrange(
                        "c hh x -> c (hh x)"),
                )
            # greedy copy-engine balance: projected busy ns per engine
            # (per-instruction init: ACT 2*222cyc/2, DVE 2*120cyc/2 busy)
            eng_ns = {"v": 0.0, "s": 0.0}
            cyc = {"v": 1.0 / 0.96, "s": 1.0 / 1.2}
            init = {"v": 120 * cyc["v"], "s": 222 * cyc["s"]}
            def mm1(ps, q, hh, kslot, ci):
                u0, U, xw0, W = CHUNKS[ci]
                nc.tensor.matmul(
                    ps[:U, 256 * kslot:256 * kslot + W],
                    Lsb[32 * q:32 * (q + 1),
                        hh * WIMG + u0:hh * WIMG + u0 + U],
                    Rsb[32 * q:32 * (q + 1),
                        hh * WIMG + xw0:hh * WIMG + xw0 + W],
                    start=True, stop=True,
                    tile_position=(32 * q, 0),
                )

            def drain(ps, sb, g0, nb, ci):
                _, U, _, W = CHUNKS[ci]
                src = ps[:U, :256 * nb].rearrange(
                    "u (g c) -> u g c", g=nb)[:, :, :W]
                dst = sb[:U, g0 * W:(g0 + nb) * W].rearrange(
                    "u (g w) -> u g w", g=nb)
                cost = {e: nb * W * cyc[e] + init[e] for e in ("v", "s")}
                e = min(("v", "s"), key=lambda x: eng_ns[x] + cost[x])
                eng_ns[e] += cost[e]
                if e == "s":
                    nc.scalar.mul(dst, src, 1.0 / C)
                else:
                    nc.vector.tensor_scalar_mul(dst, src, 1.0 / C)

            def store(sb, q, hh0, ci):
                _, U, _, W = CHUNKS[ci]
                hg = 2 * q + hh0 // HGRP
                dst_ap = stag[hg, GOFF[ci]:GOFF[ci] + U * HGRP * W]
                nc.sync.dma_start(
                    dst_ap.rearrange("(u k) -> u k", u=U), sb[:U, :])

            def rep_body():
                if not INTERLEAVE:
                    for q in range(4):
                        for hh0 in range(0, HQ, HGRP):
                            for ci, (u0, U, xw0, W) in enumerate(CHUNKS):
                                sb = st_pool.tile([128, HGRP * W], bf16,
                                                  tag="sb")
                                for g0 in range(0, HGRP, GB):
                                    nb = min(GB, HGRP - g0)
                                    ps = ps_pool.tile([128, 256 * GB], f32,
                                                      tag="ps")
                                    for k in range(nb):
                                        mm1(ps, q, hh0 + g0 + k, k, ci)
                                    drain(ps, sb, g0, nb, ci)
                                store(sb, q, hh0, ci)
                else:
                    # round-robin quadrants so each LDWEIGHTS overlaps
                    # another quadrant's MATMUL (per-subarray concurrency)
                    for hh0 in range(0, HQ, HGRP):
                        for ci, (u0, U, xw0, W) in enumerate(CHUNKS):
                            sbs = [st_pool.tile([128, HGRP * W], bf16,
                                                tag=f"sb{q}")
                                   for q in range(4)]
                            for g0 in range(0, HGRP, GB):
                                nb = min(GB, HGRP - g0)
                                pss = [ps_pool.tile([128, 256 * GB], f32,
                                                    tag=f"ps{q}")
                                       for q in range(4)]
                                for k in range(nb):
                                    for q in range(4):
                                        mm(pss[q], sbs[q], q, hh0,
                                           g0 + k, 1, ci)
                                for q in range(4):
                                    drain(pss[q], sbs[q], g0, nb, ci)
                            for q in range(4):
                                store(sbs[q], q, hh0, ci)

            if hw_loop:
                assert reps % UNROLL == 0
                with tc.For_i(0, reps // UNROLL) as _iv:
                    for _ in range(UNROLL):
                        rep_body()
            else:
                for _ in range(reps):
                    rep_body()
    nc.compile()
    _CACHE[key] = nc
    return nc


def _gather_idx():
    """GIDX[p, h, x]: flat index into stag.ravel() for output plane p."""
    if "idx" in _CACHE:
        return _CACHE["idx"]
    P_ = np.arange(2 * D)[:, None, None]
    dts = np.where(P_ < D, P_, -(P_ - D))  # signed disparity per plane
    X = np.arange(WIMG)[None, None, :]
    u = np.clip(X - dts, 0, WIMG - 1)      # [2D, 1, W]
    ci = (u >= CHUNKS[1][0]).astype(np.int64) + (u >= CHUNKS[2][0])
    u0 = np.choose(ci, [c[0] for c in CHUNKS])
    xw0 = np.choose(ci, [c[2] for c in CHUNKS])
    Wc = np.choose(ci, [c[3] for c in CHUNKS])
    off = np.choose(ci, GOFF)
    w = X - xw0
    base = off + (u - u0) * (HGRP * Wc) + w  # [2D, 1, W]
    Hh = np.arange(H)[None, :, None]
    qq, rem = Hh // HQ, Hh % HQ
    hg = 2 * qq + rem // HGRP
    g = rem % HGRP
    gidx = hg * GTOT + base + g * Wc         # [2D, H, W]
    _CACHE["idx"] = np.ascontiguousarray(gidx.astype(np.int64))
    return _CACHE["idx"]


def _assemble(stag_b):
    """stag_b: [NHG, GTOT] packed bf16 band -> out_b [2D, H, WIMG] f32"""
    idx = _gather_idx()
    flat = np.asarray(stag_b).astype(np.float32).ravel()
    o = np.take(flat, idx)
    for d in range(1, D):
        o[d, :, :d] = 0
        o[D + d, :, WIMG - d:] = 0
    return o


def run_cores(right_np, left_np, timing_reps=0):
    """Run the SPMD bass kernel; returns list of per-core staging arrays."""
    import ml_dtypes
    from concourse.bass_utils import run_bass_kernel_spmd

    nc = _get_nc()
    bf = ml_dtypes.bfloat16
    in_maps = [
        {"r_in": np.ascontiguousarray(right_np[b].astype(bf)),
         "l_in": np.ascontiguousarray(left_np[b].astype(bf))}
        for b in range(B)
    ]
    res = run_bass_kernel_spmd(nc, in_maps, list(range(B)))
    return [res.results[b]["stag"] for b in range(B)]


def kernel(right_feature, left_feature, max_disp):
    assert int(max_disp) == D
    right_np = np.asarray(right_feature, dtype=np.float32)
    left_np = np.asarray(left_feature, dtype=np.float32)
    stags = run_cores(right_np, left_np)
    out = np.stack([_assemble(s) for s in stags])
    return out


# revision 23
# speedup vs baseline: 1.1960x; 1.1960x over previous
"""Bi-directional correlation cost volume on 8 Trainium2 NeuronCores.

Strategy (data-parallel over batch, one batch element per core):
  - Host casts both feature maps to bf16 (rel err ~3e-3, gate is 2e-2).
    The PE then computes the Gram band G[u, x] = sum_c L[c,h,u] * R[c,h,x]
    at bf16 rate (1 cycle/row @2.4GHz vs fp32's 4 cycles/row), 4x
    h-quadrant row-tiled (K=32 each) via tile_position.
  - Chunk sizes (128, 64, 128): edge chunks keep exactly 128 weight
    columns so the PE's Fast Weight Load stays enabled (misaligned M like
    117 falls off the fast path and is ~4x slower end-to-end on HW),
    while sum(U_i*W_i) - the staged-byte count - is near its minimum at
    fixed sum(W_i)=572 columns of PSUM-drain work.
  - PSUM (f32) is drained to SBUF as bf16 with the 1/C scale fused in.
    Drains are batched 4 matmuls per instruction (one 2-bank PSUM tile)
    to amortize the fixed ~2*220cyc SBUF/PSUM access latency, and split
    across DVE and ACT by greedy load balance. DVE+ACT drain capacity
    (~2.16 cols/ns) and store bandwidth are the two walls; this config
    sits at the argmin of max(DMA, drain).
  - The staged band is packed [u, g, w] per (h-group, chunk) so every DMA
    descriptor moves one >=5KB contiguous run per partition row (runs
    under 512B pay a 2x DMA-latency penalty): 19.5MB/core/rep at full
    store bandwidth.
  - The cost volume out[d, x] = G[x -/+ d, x] is a *shear* of the band;
    host extracts the 127 diagonals with one vectorized gather per batch.
"""

import numpy as np

B, C, H, WIMG, D = 8, 32, 160, 320, 64
import os
# (u0, U, xw0, W): u-chunk start/size, x-window start/size.
# U split (117, 86, 117) minimizes staged bytes sum(U_i*W_i) at fixed
# sum(W_i)=572, so PE/copy cost is unchanged but stores drop ~7%.
_KCH = os.environ.get("KCH", "fwl")
if _KCH == "old":
    CHUNKS = [(0, 128, 0, 191), (128, 128, 65, 254), (256, 64, 193, 127)]
elif _KCH == "new":
    CHUNKS = [(0, 117, 0, 180), (117, 86, 54, 212), (203, 117, 140, 180)]
else:
    # M=128 edge chunks keep the PE's Fast Weight Load (needs exactly 128
    # weight columns) while staying near the min-staged-bytes optimum.
    CHUNKS = [(0, 128, 0, 191), (128, 64, 65, 190), (192, 128, 129, 191)]
HQ = H // 4      # 40 h-rows per PE quadrant
HGRP = 20        # h-rows batched per staging tile / store DMA
NHG = H // HGRP  # 8 h-groups
# per-group staging: chunk ci at GOFF[ci], laid out [u, g, w] contiguous
GOFF = [0, CHUNKS[0][1] * HGRP * CHUNKS[0][3],
        CHUNKS[0][1] * HGRP * CHUNKS[0][3]
        + CHUNKS[1][1] * HGRP * CHUNKS[1][3]]
GTOT = GOFF[2] + CHUNKS[2][1] * HGRP * CHUNKS[2][3]

_CACHE = {}

# tuning knobs (env-overridable for experiments; defaults are the config
# the grading harness runs)
GB = int(os.environ.get("KGB", "4"))         # matmuls per PSUM tile/copy
PS_BUFS = int(os.environ.get("KPSB", "4"))   # PSUM pool buffers
ST_BUFS = int(os.environ.get("KSTB", "6"))   # staging pool buffers
INTERLEAVE = int(os.environ.get("KINT", "0"))  # round-robin PE quadrants
UNROLL = int(os.environ.get("KUNR", "8"))  # reps per For_i iteration


def _get_nc(reps=1, hw_loop=False):
    """reps identical kernel bodies; with hw_loop, a For_i loop of
    reps//UNROLL iterations around an UNROLL-times unrolled body (constant
    NEFF size, so huge rep counts stay compilable — used for timing)."""
    key = ("nc", reps, GB, PS_BUFS, ST_BUFS, CHUNKS[0][1], INTERLEAVE,
           hw_loop, UNROLL)
    if key in _CACHE:
        return _CACHE[key]
    import concourse.bacc as bacc
    import concourse.tile as tile
    from concourse import mybir

    f32 = mybir.dt.float32
    bf16 = mybir.dt.bfloat16
    nc = bacc.Bacc("TRN2", target_bir_lowering=False, debug=False)
    r_in = nc.declare_dram_parameter("r_in", [C, H, WIMG], bf16, isOutput=False)
    l_in = nc.declare_dram_parameter("l_in", [C, H, WIMG], bf16, isOutput=False)
    stag = nc.declare_dram_parameter("stag", [NHG, GTOT], bf16, isOutput=True)

    with tile.TileContext(nc) as tc:
        with tc.tile_pool(name="inp", bufs=1) as inp_pool, \
             tc.tile_pool(name="ps", bufs=PS_BUFS, space="PSUM") as ps_pool, \
             tc.tile_pool(name="st", bufs=ST_BUFS) as st_pool:
            Lsb = inp_pool.tile([128, HQ * WIMG], bf16, tag="L")
            Rsb = inp_pool.tile([128, HQ * WIMG], bf16, tag="R")
            # partition (q, c) holds h-rows [40q, 40q+40) of channel c
            for q in range(4):
                nc.sync.dma_start(
                    Lsb[32 * q:32 * (q + 1), :],
                    l_in[:, HQ * q:HQ * (q + 1), :].rearrange(
                        "c hh x -> c (hh x)"),
                )
                nc.sync.dma_start(
                    Rsb[32 * q:32 * (q + 1), :],
                    r_in[:, HQ * q:HQ * (q + 1), :].rearrange(
                        "c hh x -> c (hh x)"),
                )
            # greedy copy-engine balance: projected busy ns per engine
            # (per-instruction init: ACT 2*222cyc/2, DVE 2*120cyc/2 busy)
            eng_ns = {"v": 0.0, "s": 0.0}
            cyc = {"v": 1.0 / 0.96, "s": 1.0 / 1.2}
            init = {"v": 120 * cyc["v"], "s": 222 * cyc["s"]}
            def mm1(ps, q, hh, kslot, ci):
                u0, U, xw0, W = CHUNKS[ci]
                nc.tensor.matmul(
                    ps[:U, 256 * kslot:256 * kslot + W],
                    Lsb[32 * q:32 * (q + 1),
                        hh * WIMG + u0:hh * WIMG + u0 + U],
                    Rsb[32 * q:32 * (q + 1),
                        hh * WIMG + xw0:hh * WIMG + xw0 + W],
                    start=True, stop=True,
                    tile_position=(32 * q, 0),
                )

            def drain(ps, sb, g0, nb, ci):
                _, U, _, W = CHUNKS[ci]
                src = ps[:U, :256 * nb].rearrange(
                    "u (g c) -> u g c", g=nb)[:, :, :W]
                dst = sb[:U, g0 * W:(g0 + nb) * W].rearrange(
                    "u (g w) -> u g w", g=nb)
                cost = {e: nb * W * cyc[e] + init[e] for e in ("v", "s")}
                e = min(("v", "s"), key=lambda x: eng_ns[x] + cost[x])
                eng_ns[e] += cost[e]
                if e == "s":
                    nc.scalar.mul(dst, src, 1.0 / C)
                else:
                    nc.vector.tensor_scalar_mul(dst, src, 1.0 / C)

            def store(sb, q, hh0, ci):
                _, U, _, W = CHUNKS[ci]
                hg = 2 * q + hh0 // HGRP
                dst_ap = stag[hg, GOFF[ci]:GOFF[ci] + U * HGRP * W]
                nc.sync.dma_start(
                    dst_ap.rearrange("(u k) -> u k", u=U), sb[:U, :])

            def rep_body():
                if not INTERLEAVE:
                    for q in range(4):
                        for hh0 in range(0, HQ, HGRP):
                            for ci, (u0, U, xw0, W) in enumerate(CHUNKS):
                                sb = st_pool.tile([128, HGRP * W], bf16,
                                                  tag="sb")
                                for g0 in range(0, HGRP, GB):
                                    nb = min(GB, HGRP - g0)
                                    ps = ps_pool.tile([128, 256 * GB], f32,
                                                      tag="ps")
                                    for k in range(nb):
                                        mm1(ps, q, hh0 + g0 + k, k, ci)
                                    drain(ps, sb, g0, nb, ci)
                                store(sb, q, hh0, ci)
                else:
                    # round-robin quadrants so each LDWEIGHTS overlaps
                    # another quadrant's MATMUL (per-subarray concurrency)
                    for hh0 in range(0, HQ, HGRP):
                        for ci, (u0, U, xw0, W) in enumerate(CHUNKS):
                            sbs = [st_pool.tile([128, HGRP * W], bf16,
                                                tag=f"sb{q}")
                                   for q in range(4)]
                            for g0 in range(0, HGRP, GB):
                                nb = min(GB, HGRP - g0)
                                pss = [ps_pool.tile([128, 256 * GB], f32,
                                                    tag=f"ps{q}")
                                       for q in range(4)]
                                for k in range(nb):
                                    for q in range(4):
                                        mm(pss[q], sbs[q], q, hh0,
                                           g0 + k, 1, ci)
                                for q in range(4):
                                    drain(pss[q], sbs[q], g0, nb, ci)
                            for q in range(4):
                                store(sbs[q], q, hh0, ci)

            if hw_loop:
                assert reps % UNROLL == 0
                with tc.For_i(0, reps // UNROLL) as _iv:
                    for _ in range(UNROLL):
                        rep_body()
            else:
                for _ in range(reps):
                    rep_body()
    nc.compile()
    _CACHE[key] = nc
    return nc


def _gather_idx():
    """GIDX[p, h, x]: flat index into stag.ravel() for output plane p."""
    if "idx" in _CACHE:
        return _CACHE["idx"]
    P_ = np.arange(2 * D)[:, None, None]
    dts = np.where(P_ < D, P_, -(P_ - D))  # signed disparity per plane
    X = np.arange(WIMG)[None, None, :]
    u = np.clip(X - dts, 0, WIMG - 1)      # [2D, 1, W]
    ci = (u >= CHUNKS[1][0]).astype(np.int64) + (u >= CHUNKS[2][0])
    u0 = np.choose(ci, [c[0] for c in CHUNKS])
    xw0 = np.choose(ci, [c[2] for c in CHUNKS])
    Wc = np.choose(ci, [c[3] for c in CHUNKS])
    off = np.choose(ci, GOFF)
    w = X - xw0
    base = off + (u - u0) * (HGRP * Wc) + w  # [2D, 1, W]
    Hh = np.arange(H)[None, :, None]
    qq, rem = Hh // HQ, Hh % HQ
    hg = 2 * qq + rem // HGRP
    g = rem % HGRP
    gidx = hg * GTOT + base + g * Wc         # [2D, H, W]
    _CACHE["idx"] = np.ascontiguousarray(gidx.astype(np.int64))
    return _CACHE["idx"]


def _assemble(stag_b):
    """stag_b: [NHG, GTOT] packed bf16 band -> out_b [2D, H, WIMG] f32"""
    idx = _gather_idx()
    flat = np.asarray(stag_b).astype(np.float32).ravel()
    o = np.take(flat, idx)
    for d in range(1, D):
        o[d, :, :d] = 0
        o[D + d, :, WIMG - d:] = 0
    return o


def run_cores(right_np, left_np, timing_reps=0):
    """Run the SPMD bass kernel; returns list of per-core staging arrays."""
    import ml_dtypes
    from concourse.bass_utils import run_bass_kernel_spmd

    nc = _get_nc()
    bf = ml_dtypes.bfloat16
    in_maps = [
        {"r_in": np.ascontiguousarray(right_np[b].astype(bf)),
         "l_in": np.ascontiguousarray(left_np[b].astype(bf))}
        for b in range(B)
    ]
    res = run_bass_kernel_spmd(nc, in_maps, list(range(B)))
    return [res.results[b]["stag"] for b in range(B)]


def kernel(right_feature, left_feature, max_disp):
    assert int(max_disp) == D
    right_np = np.asarray(right_feature, dtype=np.float32)
    left_np = np.asarray(left_feature, dtype=np.float32)
    stags = run_cores(right_np, left_np)
    out = np.stack([_assemble(s) for s in stags])
    return out
